# revision 1
# baseline (speedup 1.0000x reference)
"""BitLinear kernel for Trainium2, 8 NeuronCores, column-parallel.

y[t, o] = sum_i x[t, i] * sign(W[o, i]) * scale[o]
  x: [8192, 4096] f32 (replicated), W: [16384, 4096] f32, scale: [16384] f32
  Each core owns OUT_F/8 = 2048 output features (column parallel).

Per-core pipeline (all math on device):
  - scale_sb:  scale shard -> SBUF [128, 16]
  - W prep:    load W rows f32 -> ScalarE Sign -> f16 -> *scale (DVE broadcast)
               -> PE transpose (128x128 tiles, grouped 4/psum-bank)
               -> B resident [128, 32k, 2048n] f16  (sign*scale exact in f16)
  - m loop:    x row-tile f32 --SWDGE cast-DMA--> SBUF f16 [128, 4096]
               -> PE transpose -> xT [128, 32k, 128t] f16 (ScalarE copies)
               -> 4 x (32 fp16 matmuls accumulating K into PSUM [128, 512] f32)
               -> DVE copy -> y stage -> DMA out
fp16 keeps sign*scale exact; only x quantizes (~3e-4 rel err), PSUM is f32.
"""

import os
import sys

for _p in ("/opt/trn_rl_repo",):
    if _p not in sys.path and os.path.isdir(_p):
        sys.path.append(_p)

import numpy as np
import concourse.bacc as bacc
import concourse.mybir as mybir
from concourse.tile import TileContext
from concourse.masks import make_identity
from concourse.bass_utils import run_bass_kernel_spmd

TOKENS, IN_F, OUT_F, NCORES = 8192, 4096, 16384, 8
O_SH = OUT_F // NCORES  # 2048 out features per core
P = 128
KT = IN_F // P          # 32 k-subtiles
MT = TOKENS // P        # 64 token tiles
OT = O_SH // P          # 16 o-tiles per core
W_KC = 2048             # W prep free-dim chunk
NCH = O_SH // 512       # 4 psum c-chunks per token tile

f32, f16 = mybir.dt.float32, mybir.dt.float16
AF = mybir.ActivationFunctionType

_CACHE = {}
last_result = None


def build():
    nc = bacc.Bacc("TRN2", target_bir_lowering=False, debug=False)
    x = nc.dram_tensor("x", [TOKENS, IN_F], f32, kind="ExternalInput").ap()
    w = nc.dram_tensor("weight", [O_SH, IN_F], f32, kind="ExternalInput").ap()
    scale = nc.dram_tensor("scale", [O_SH], f32, kind="ExternalInput").ap()
    y = nc.dram_tensor("y", [TOKENS, O_SH], f32, kind="ExternalOutput").ap()

    with TileContext(nc) as tc:
        with (
            tc.tile_pool(name="const", bufs=1) as cpool,
            tc.tile_pool(name="bres", bufs=1) as bpool,
            tc.tile_pool(name="wstage", bufs=2) as wpool,
            tc.tile_pool(name="xstage", bufs=2) as xpool,
            tc.tile_pool(name="xtp", bufs=2) as xtpool,
            tc.tile_pool(name="ystage", bufs=4) as ypool,
            tc.tile_pool(name="mmps", bufs=5, space="PSUM") as mmps,
            tc.tile_pool(name="tpps", bufs=3, space="PSUM") as tpps,
        ):
            ident = cpool.tile([P, P], f16, tag="ident")
            make_identity(nc, ident)
            scale_sb = cpool.tile([P, OT], f32, tag="scale")
            nc.sync.dma_start(scale_sb[:], scale.rearrange("(o p) -> p o", p=P))

            # ---------------- W prep -> B [P, KT, O_SH] f16 ----------------
            B = bpool.tile([P, KT, O_SH], f16, tag="B")
            for ot in range(OT):
                for kc in range(IN_F // W_KC):  # 2 chunks of 2048
                    ws = wpool.tile([P, W_KC], f32, tag="ws")
                    nc.sync.dma_start(
                        ws[:], w[ot * P : (ot + 1) * P, kc * W_KC : (kc + 1) * W_KC]
                    )
                    wsg = wpool.tile([P, W_KC], f16, tag="wsg")
                    nc.scalar.activation(wsg[:], ws[:], AF.Sign)
                    nc.vector.tensor_tensor(
                        wsg[:],
                        wsg[:],
                        scale_sb[:, ot : ot + 1].to_broadcast((P, W_KC)),
                        mybir.AluOpType.mult,
                    )
                    ksub0 = kc * (W_KC // P)  # 16 k-subtiles per chunk
                    for g in range(W_KC // P // 4):  # 4 groups of 4
                        tp = tpps.tile([P, 512], f16, tag="tp")
                        for j in range(4):
                            ki = g * 4 + j
                            nc.tensor.transpose(
                                tp[:, j * P : (j + 1) * P],
                                wsg[:, ki * P : (ki + 1) * P],
                                ident[:],
                            )
                        k0 = ksub0 + g * 4
                        nc.vector.tensor_copy(
                            B[:, k0 : k0 + 4, ot * P : (ot + 1) * P],
                            tp[:].rearrange("p (a b) -> p a b", a=4),
                        )

            # ---------------- main loop over 64 token tiles ----------------
            for mt in range(MT):
                xc = xpool.tile([P, IN_F], f16, tag="xc")
                # SWDGE casting DMA: f32 DRAM -> f16 SBUF
                nc.gpsimd.dma_start(xc[:], x[mt * P : (mt + 1) * P, :])

                xT = xtpool.tile([P, KT, P], f16, tag="xT")
                for g in range(KT // 4):  # 8 groups of 4 transposes
                    tp = tpps.tile([P, 512], f16, tag="tp")
                    for j in range(4):
                        ki = g * 4 + j
                        nc.tensor.transpose(
                            tp[:, j * P : (j + 1) * P],
                            xc[:, ki * P : (ki + 1) * P],
                            ident[:],
                        )
                    nc.scalar.activation(
                        xT[:, g * 4 : g * 4 + 4, :],
                        tp[:].rearrange("p (a b) -> p a b", a=4),
                        AF.Copy,
                    )

                for h in range(2):  # two 1024-wide output halves
                    ystage = ypool.tile([P, 1024], f32, tag="ystage")
                    for c in range(2):
                        ps = mmps.tile([P, 512], f32, tag="ps")
                        n0 = h * 1024 + c * 512
                        for k in range(KT):
                            nc.tensor.matmul(
                                ps[:],
                                xT[:, k, :],
                                B[:, k, n0 : n0 + 512],
                                start=(k == 0),
                                stop=(k == KT - 1),
                            )
                        nc.vector.tensor_copy(
                            ystage[:, c * 512 : (c + 1) * 512], ps[:]
                        )
                    nc.sync.dma_start(
                        y[mt * P : (mt + 1) * P, h * 1024 : (h + 1) * 1024],
                        ystage[:],
                    )

    nc.finalize()
    return nc


def _get_nc():
    if "nc" not in _CACHE:
        _CACHE["nc"] = build()
    return _CACHE["nc"]


def kernel(x, weight, scale):
    global last_result
    nc = _get_nc()
    x = np.ascontiguousarray(np.asarray(x, dtype=np.float32))
    weight = np.ascontiguousarray(np.asarray(weight, dtype=np.float32))
    scale = np.ascontiguousarray(np.asarray(scale, dtype=np.float32))
    in_maps = [
        {
            "x": x,
            "weight": np.ascontiguousarray(weight[c * O_SH : (c + 1) * O_SH]),
            "scale": np.ascontiguousarray(scale[c * O_SH : (c + 1) * O_SH]),
        }
        for c in range(NCORES)
    ]
    res = run_bass_kernel_spmd(nc, in_maps, list(range(NCORES)))
    last_result = res
    return np.concatenate([res.results[c]["y"] for c in range(NCORES)], axis=1)


if __name__ == "__main__":
    rng = np.random.default_rng(0)
    xv = rng.standard_normal((TOKENS, IN_F), dtype=np.float32)
    wv = rng.standard_normal((OUT_F, IN_F), dtype=np.float32)
    sv = np.ones(OUT_F, dtype=np.float32)
    yv = kernel(xv, wv, sv)
    print("out shape:", yv.shape, yv.dtype)
